# revision 29
# baseline (speedup 1.0000x reference)
"""Trainium2 Bass kernel for a 6-layer transformer decoder (nn_Decoder).

Sharding: data-parallel over batch across 8 NeuronCores (2 sequences/core,
weights replicated).  Everything heavy (subsampling FFN, 6 decoder layers:
self-attn, cross-attn, FFN, layernorms, softmaxes) runs on device; the host
only does the embedding-table gather + positional-encoding add, computes the
tiny pad-keep masks / causal mask constants, and folds the 1/sqrt(dk) scale
into the query projection weights.

Device-side layout strategy:
  - activations kept in BOTH token-major [tok, D] (for residual + layernorm)
    and feature-major transposed [D, tok] (as matmul contraction operands).
  - attention computed with TRANSPOSED scores s^T[k, q] so the softmax
    numerator (exp) needs no transposes at all; the softmax denominator is
    produced by an extra ones-column appended to V in the ctx matmul; pad
    masking is applied by zeroing V rows (and the ones column) with a
    per-partition keep-scale during the V copy; causal masking is one
    vector-engine add of a resident causal^T tile.
"""

import sys

import numpy as np

# ---------------------------------------------------------------- constants
B, S, D, H, FF, L, V = 16, 512, 512, 8, 2048, 6, 32000
NCORES = 8
BPC = B // NCORES          # batches per core
T = BPC * S                # tokens per core
P = 128
DK = D // H                # 64
NT = T // P                # 8 token tiles per core
NTB = S // P               # 4 token tiles per batch
NF = D // P                # 4 feature tiles
NHC = 4                    # hidden chunks in FFN (each FF/NHC = 512 wide)
EPS = 1e-5

_REPO = "/opt/trn_rl_repo"


def _ensure_env():
    if _REPO not in sys.path:
        sys.path.insert(0, _REPO)
    import concourse.mybir as mybir
    import concourse.tile as tile
    from bass_rust import ScopedClock

    # This container's walrus build only accepts ONE sync-wait per CTRL
    # instruction; the stock TileContext tail drain aggregates all residual
    # clock waits onto a single Drain.  Split them across multiple drains.
    if not getattr(tile.TileContext, "_ant_drain_patched", False):

        def _drain_and_barrier(self, tick_clock, wait_clock):
            nc = self.nc
            probe = nc.sync.drain()
            wait_clock.add_sem_waits(
                probe.ins, ScopedClock({None: tick_clock.global_clock})
            )
            si = probe.ins.sync_info
            waits = list(si.on_wait) if si is not None else []
            if len(waits) > 1:
                probe.ins.sync_info = mybir.SyncInfo(
                    on_wait=[waits[0]], on_update=list(si.on_update)
                )
                for w in waits[1:]:
                    extra = nc.sync.drain()
                    extra.ins.sync_info = mybir.SyncInfo(on_wait=[w], on_update=[])
            nc.all_engine_barrier()
            popped = nc._tile_sem_poison_stack.pop()
            assert popped is self._sem_poison
            nc.clear_and_free_semaphores(list(self.sems.allocated().values()))
            nc.all_engine_barrier()

        tile.TileContext._drain_and_barrier = _drain_and_barrier
        tile.TileContext._ant_drain_patched = True


def _split_multi_waits(nc):
    """This container's walrus build accepts only ONE sync-wait per
    instruction.  Hoist extra waits onto single-wait NoOps inserted just
    before the instruction on the same engine (per-engine program order is
    what the sequencers execute, so the gating semantics are identical)."""
    import concourse.mybir as mybir

    n = 0
    for f in nc.m.functions:
        for b in f.blocks:
            insts = list(b.instructions)
            out = []
            changed = False
            for inst in insts:
                si = inst.sync_info
                if si is not None and len(si.on_wait) > 1:
                    waits = list(si.on_wait)
                    for k, w in enumerate(waits[:-1]):
                        nop = mybir.InstNoOp(name=f"{inst.name}_sw{k}",
                                             engine=inst.engine, ins=[], outs=[])
                        nop.sync_info = mybir.SyncInfo(on_wait=[w], on_update=[])
                        out.append(nop)
                        n += 1
                    inst.sync_info = mybir.SyncInfo(
                        on_wait=[waits[-1]], on_update=list(si.on_update))
                    changed = True
                out.append(inst)
            if changed:
                b.instructions = out
    return n


# ------------------------------------------------------------------ builder
def _build(mm_dt="f32r", flags=None):
    """Emit the Bass program.  Returns (nc, input_names)."""
    _ensure_env()
    import concourse.bass as bass
    import concourse.mybir as mybir
    import concourse.tile as tile
    fl = flags or {}
    F32 = mybir.dt.float32
    # float32r = single-pass fp32 matmul mode (4x faster than fp32 on the PE
    # at N>=512, TF32-like rounding of the multiplicands, fp32 accumulate).
    # The BIR verifier requires every producer of an fp32r matmul operand to
    # declare fp32r output, so matmul-feeding tensors are typed MMDT
    # end-to-end.
    MMDT = mybir.dt.float32r if mm_dt == "f32r" else F32

    def mmv(ap):
        return ap

    nc = bass.Bass()

    # ---------------- dram tensors
    x0_d = nc.dram_tensor("x0", [T, D], F32, kind="ExternalInput")
    ident_d = nc.dram_tensor("ident", [P, P], F32, kind="ExternalInput")
    ones_d = nc.dram_tensor("ones", [1, P], MMDT, kind="ExternalInput")
    sel8_d = nc.dram_tensor("sel8", [P, 2 * P], MMDT, kind="ExternalInput")
    encT_d = nc.dram_tensor("encT", [P, NF, T], MMDT, kind="ExternalInput")
    causal_d = nc.dram_tensor("causalT", [P, NTB, S], mybir.dt.bfloat16,
                              kind="ExternalInput")
    keep_dec_d = nc.dram_tensor("keep_dec", [P, NT], F32, kind="ExternalInput")
    keep_enc_d = nc.dram_tensor("keep_enc", [P, NT], F32, kind="ExternalInput")
    out_d = nc.dram_tensor("out", [T, D], F32, kind="ExternalOutput")

    wname = []
    wd = {}

    def dram_w(name, shape):
        wd[name] = nc.dram_tensor(name, shape, MMDT, kind="ExternalInput")
        wname.append(name)

    dram_w("sub_w1", [D, FF]); dram_w("sub_w2", [FF, D])
    for l in range(L):
        for pre in ("sa", "ca"):
            for wn in ("wq", "wk", "wv", "wo"):
                dram_w(f"{pre}_{wn}_{l}", [D, D])
        dram_w(f"ff_w1_{l}", [D, FF]); dram_w(f"ff_w2_{l}", [FF, D])
    if fl.get("any_rows"):
        dram_w("rows", [fl["n_rows"], D])
    if fl.get("any_cols"):
        wd["cols"] = nc.dram_tensor("cols", [P, fl["n_cols"]], F32, kind="ExternalInput")
        wname.append("cols")

    AF = mybir.ActivationFunctionType
    OP = mybir.AluOpType

    with tile.TileContext(nc) as tc:
        cpool = tc.alloc_tile_pool(name="consts", bufs=1)
        pool = tc.alloc_tile_pool(name="work", bufs=2)
        spool = tc.alloc_tile_pool(name="stats", bufs=8)
        ppool = tc.alloc_tile_pool(name="ps", bufs=2, space="PSUM")

        ident = cpool.tile([P, P], F32, name="ident_sb")
        nc.sync.dma_start(ident[:], ident_d[:])
        causalT = cpool.tile([P, NTB, S], mybir.dt.bfloat16, name="causalT")
        nc.sync.dma_start(causalT[:], causal_d[:])
        keep_dec = cpool.tile([P, NT], F32, name="keep_dec_sb")
        nc.sync.dma_start(keep_dec[:], keep_dec_d[:])
        keep_enc = cpool.tile([P, NT], F32, name="keep_enc_sb")
        nc.sync.dma_start(keep_enc[:], keep_enc_d[:])
        ones_row = cpool.tile([1, P], MMDT, name="ones_row")
        nc.sync.dma_start(ones_row[:], ones_d[:])
        sel8 = cpool.tile([P, 2 * P], MMDT, name="sel8_sb")
        nc.sync.dma_start(sel8[:], sel8_d[:])
        eps_col = cpool.tile([P, 1], F32, name="eps_col")
        nc.vector.memset(eps_col[:], EPS)
        rows_sb = cols_sb = None
        if fl.get("any_rows"):
            rows_sb = cpool.tile([fl["n_rows"], D], MMDT, name="rows_sb")
            nc.sync.dma_start(rows_sb[:], wd["rows"][:])
        if fl.get("any_cols"):
            cols_sb = cpool.tile([P, fl["n_cols"]], F32, name="cols_sb")
            nc.sync.dma_start(cols_sb[:], wd["cols"][:])

        def load_w_proj(name):
            w = pool.tile([P, NF, D], MMDT, tag="w", bufs=3, name="w_" + name)
            nc.sync.dma_start(w[:], wd[name].rearrange("(k p) d -> p k d", p=P))
            return w

        def psum(shape, tag, name):
            # one uniform 2-bank slot class, 4 slots = all 8 banks; deeper
            # rotation for the hot matmul pipelines
            return ppool.tile(shape, F32, tag="S", bufs=4, name=name)

        def seed_or_start(ps, row_idx, kt):
            """Start flag for the kt-th accumulation matmul; optionally seed
            the psum with a broadcast bias row first (K=1 matmul)."""
            if row_idx is not None and kt == 0:
                nc.tensor.matmul(
                    ps, mmv(ones_row[0:1, 0:ps.shape[0]]),
                    mmv(rows_sb[row_idx:row_idx + 1, :ps.shape[-1]]),
                    start=True, stop=False)
                return False
            return kt == 0

        # ---------------- building blocks --------------------------------
        def transpose_b(x_sb, b, name):
            """token-major x tile [P, NT, D] (batch b) -> feature-major
            xT_b [P, NF, S]."""
            xT = pool.tile([P, NF, S], MMDT, tag="xT", bufs=2, name=name)
            for t in range(NTB):
                ps = psum([P, 512], "C", f"{name}_ps{t}")
                for f in range(NF):
                    nc.tensor.transpose(
                        ps[:, f * P:(f + 1) * P],
                        x_sb[:, b * NTB + t, f * P:(f + 1) * P],
                        ident[:],
                    )
                nc.vector.tensor_copy(
                    out=xT[:, :, t * P:(t + 1) * P],
                    in_=ps.rearrange("p (f j) -> p f j", j=P),
                )
            return xT

        def proj_featmajor(srcT, w, tag, name, col_base=None):
            """y^T = (src @ w)^T: [P, NF, S] feature-major output.  Two
            output feature tiles share one 2-bank psum for deeper PE
            pipelining and halved copy-out count."""
            yT = pool.tile([P, NF, S], MMDT, tag=tag, bufs=1, name=name)
            for fp in range(NF // 2):
                ps = psum([P, 1024], "S", f"{name}_ps{fp}")
                for j in range(2):
                    f = fp * 2 + j
                    for kt in range(NF):
                        nc.tensor.matmul(
                            ps[:, j * 512:(j + 1) * 512],
                            mmv(w[:, kt, f * P:(f + 1) * P]), mmv(srcT[:, kt, :]),
                            start=(kt == 0), stop=(kt == NF - 1))
                if col_base is not None:
                    for j in range(2):
                        f = fp * 2 + j
                        nc.scalar.activation(
                            yT[:, f, :], ps[:, j * 512:(j + 1) * 512], AF.Identity,
                            bias=cols_sb[:, col_base + f:col_base + f + 1])
                else:
                    nc.scalar.copy(yT[:, fp * 2:fp * 2 + 2, :],
                                   ps.rearrange("p (j q) -> p j q", q=512))
            return yT

        def proj_v(srcT, w, keep, b, name, row_idx=None):
            """token-major V with interleaved ones-columns, rows scaled by
            keep (pad masking).  [P, NTB, H*65] for batch b."""
            v = pool.tile([P, NTB, H * 65], MMDT, tag="v", bufs=1, name=name)
            for t in range(NTB):
                ps = psum([P, 512], "C", f"{name}_ps{t}")
                for kt in range(NF):
                    st = seed_or_start(ps, row_idx, kt)
                    nc.tensor.matmul(
                        ps, mmv(srcT[:, kt, t * P:(t + 1) * P]), mmv(w[:, kt, :]),
                        start=st, stop=(kt == NF - 1))
                kcol = keep[:, b * NTB + t:b * NTB + t + 1]
                vv = v[:, t, :].rearrange("p (h c) -> p h c", c=65)
                nc.scalar.activation(vv[:, :, 0:64],
                                     ps.rearrange("p (h c) -> p h c", c=64),
                                     AF.Copy, scale=kcol)
                nc.vector.tensor_copy(out=vv[:, :, 64:65],
                                      in_=kcol.to_broadcast((P, H, 1)))
            return v

        def ln_chain(ps, x_old, x_new, tt, step, name):
            """x_new[tt] = LN(ps + x_old[tt]) with fused reductions."""
            ssum = spool.tile([P, 1], F32, tag="st", bufs=8, name=name + "_s")
            nc.vector.tensor_add(out=x_new[:, tt, :], in0=ps,
                                 in1=x_old[:, tt, :])
            # row-sum for the mean, riding along an ACT copy into the (dead)
            # psum tile
            nc.scalar.activation(ps, x_new[:, tt, :], AF.Copy,
                                 accum_out=ssum[:])
            negm = spool.tile([P, 1], F32, tag="st", bufs=8, name=name + "_m")
            nc.scalar.mul(negm[:], ssum[:], -1.0 / D)
            var = spool.tile([P, 1], F32, tag="st", bufs=8, name=name + "_v")
            nc.scalar.activation(ps, x_new[:, tt, :], AF.Square,
                                 bias=negm[:], accum_out=var[:])
            std = spool.tile([P, 1], F32, tag="st", bufs=8, name=name + "_d")
            nc.scalar.activation(std[:], var[:], AF.Sqrt,
                                 bias=eps_col[:], scale=1.0 / D)
            rstd = spool.tile([P, 1], F32, tag="st", bufs=8, name=name + "_r")
            nc.vector.reciprocal(rstd[:], std[:])
            nc.vector.tensor_scalar(
                out=x_new[:, tt, :], in0=x_new[:, tt, :],
                scalar1=negm[:], scalar2=rstd[:], op0=OP.add, op1=OP.mult)
            gi = fl.get(f"g_{step}")
            if gi is not None:
                gb = _row_bcast(gi, name + "_g")
                nc.vector.tensor_mul(out=x_new[:, tt, :],
                                     in0=x_new[:, tt, :], in1=gb[0:P, :])
            bi = fl.get(f"beta_{step}")
            if bi is not None:
                bb = _row_bcast(bi, name + "_b")
                nc.vector.tensor_add(out=x_new[:, tt, :],
                                     in0=x_new[:, tt, :], in1=bb[0:P, :])

        def _row_bcast(row_idx, name):
            ps = psum([P, 512], "C", name + "_ps")
            nc.tensor.matmul(ps, mmv(ones_row[0:1, 0:P]),
                             mmv(rows_sb[row_idx:row_idx + 1, :]),
                             start=True, stop=True)
            sb = pool.tile([P, 512], F32, tag="gb", bufs=2, name=name + "_sb")
            nc.scalar.copy(sb[:], ps)
            return sb

        def attention(x_sb, x_new, wq_n, wk_n, wv_n, wo_n, cross, l):
            """One attention sub-block (self or cross) + residual + LN."""
            tagp = "cross" if cross else "self"
            wq = load_w_proj(wq_n); wk = load_w_proj(wk_n)
            wv = load_w_proj(wv_n)
            keep = keep_enc if cross else keep_dec
            D_qT, D_kT, D_vv, D_ctxT, D_dns = {}, {}, {}, {}, {}

            def emit_prep(b):
                xT = transpose_b(x_sb, b, f"xT_{tagp}_{l}_{b}")
                if cross:
                    srcT = pool.tile([P, NF, S], MMDT, tag="encT", bufs=1,
                                     name=f"encTs_{l}_{b}")
                    nc.sync.dma_start(srcT[:], encT_d[:, :, b * S:(b + 1) * S])
                else:
                    srcT = xT
                D_qT[b] = proj_featmajor(xT, wq, "qT", f"qT_{tagp}_{l}_{b}",
                                         col_base=fl.get(f"bq_{tagp}_{l}"))
                D_kT[b] = proj_featmajor(srcT, wk, "kT", f"kT_{tagp}_{l}_{b}",
                                         col_base=fl.get(f"bk_{tagp}_{l}"))
                D_vv[b] = proj_v(srcT, wv, keep, b, f"v_{tagp}_{l}_{b}",
                                 row_idx=fl.get(f"bv_{tagp}_{l}"))
                D_ctxT[b] = pool.tile([P, NF, S], MMDT, tag="ctxT", bufs=2,
                                      name=f"ctxT_{tagp}_{l}_{b}")
                dns = []
                for g in range(2):
                    dng = pool.tile([P, 512], F32, tag="dncol", bufs=4,
                                    name=f"dncol_{tagp}_{l}_{b}_{g}")
                    nc.vector.memset(dng[:], 1.0)
                    dns.append(dng)
                D_dns[b] = dns
            def emit_scores(b, h):
                qT, kT = D_qT[b], D_kT[b]
                po = (h % 2) * 64
                ft = h // 2
                expT = pool.tile([P, NTB, S], MMDT, tag="expT", bufs=2,
                                 name=f"expT_{tagp}_{l}_{b}_{h}")
                for half in range(2):
                    ps = psum([P, 1024], "S", f"s_{tagp}_{l}_{b}_{h}_{half}")
                    for j in range(2):
                        kt = half * 2 + j
                        nc.tensor.matmul(
                            ps[:, j * 512:(j + 1) * 512],
                            mmv(kT[po:po + 64, ft, kt * P:(kt + 1) * P]),
                            mmv(qT[po:po + 64, ft, :]),
                            start=True, stop=True)
                    psv = ps.rearrange("p (j q) -> p j q", q=512)
                    if not cross:
                        nc.vector.tensor_tensor(
                            out=psv, in0=psv,
                            in1=causalT[:, half * 2:half * 2 + 2, :],
                            op=OP.add)
                    nc.scalar.activation(
                        expT[:, half * 2:half * 2 + 2, :], psv, AF.Exp)
                return expT

            ctx_ps = {}

            def emit_ctx(b, h, expT):
                vv, ctxT, dns = D_vv[b], D_ctxT[b], D_dns[b]
                po = (h % 2) * 64
                ft = h // 2
                if h % 2 == 0:
                    ctx_ps[b] = psum([P, 1024], "S", f"c2_{tagp}_{l}_{b}_{h}")
                psc = ctx_ps[b][:, (h % 2) * 512:(h % 2 + 1) * 512]
                for kt in range(NTB):
                    nc.tensor.matmul(
                        psc[0:65, :],
                        mmv(vv[:, kt, h * 65:h * 65 + 65]),
                        mmv(expT[:, kt, :]),
                        start=(kt == 0), stop=(kt == NTB - 1))
                ro = 32 * (h % 4)
                nc.scalar.copy(dns[h // 4][ro:ro + 1, :], psc[64:65, :])
                nc.vector.tensor_copy(out=ctxT[po:po + 64, ft, :],
                                      in_=psc[0:64, :])

            # software-pipelined over (h, b): the in-order PE always has
            # independent scores work while exp/copies of the previous
            # (h, b) pair complete on the scalar/vector engines
            for b in range(BPC):
                emit_prep(b)
                prev = None
                for h in range(H):
                    e = emit_scores(b, h)
                    if prev is not None:
                        emit_ctx(*prev)
                    prev = (b, h, e)
                emit_ctx(*prev)
            # batched reciprocals, head-pair broadcast + in-place normalize
            D_rcs = {}
            for b in range(BPC):
                rcs = []
                for g in range(2):
                    rcg = pool.tile([P, 512], MMDT, tag="rc", bufs=4,
                                    name=f"rc_{tagp}_{l}_{b}_{g}")
                    with nc.allow_low_precision(reason="fp32r matmul feed"):
                        nc.vector.reciprocal(rcg[:], D_dns[b][g][:])
                    rcs.append(rcg)
                D_rcs[b] = rcs
            for b in range(BPC):
                for hp in range(H // 2):
                    psr = psum([P, 512], "C", f"rb_{tagp}_{l}_{b}_{hp}")
                    j = hp % 2
                    nc.tensor.matmul(psr, mmv(sel8[:, j * P:(j + 1) * P]),
                                     mmv(D_rcs[b][hp // 2][:]),
                                     start=True, stop=True)
                    nc.vector.tensor_mul(out=D_ctxT[b][:, hp, :],
                                         in0=D_ctxT[b][:, hp, :],
                                         in1=psr[:])
            # output projection + residual + LN, token-tile pairs
            wo = load_w_proj(wo_n)
            row_idx = fl.get(f"bo_{tagp}_{l}")
            for b in range(BPC):
                ctxT = D_ctxT[b]
                for tp in range(NTB // 2):
                    ps = psum([P, 1024], "S", f"o_{tagp}_{l}_{b}_{tp}")
                    for j in range(2):
                        t = tp * 2 + j
                        half = ps[:, j * 512:(j + 1) * 512]
                        for ft2 in range(NF):
                            st = seed_or_start(half, row_idx, ft2)
                            nc.tensor.matmul(
                                half, mmv(ctxT[:, ft2, t * P:(t + 1) * P]),
                                mmv(wo[:, ft2, :]),
                                start=st, stop=(ft2 == NF - 1))
                    for j in range(2):
                        t = tp * 2 + j
                        ln_chain(ps[:, j * 512:(j + 1) * 512], x_sb, x_new,
                                 b * NTB + t, f"{tagp}_{l}",
                                 f"ln_{tagp}_{l}_{b}_{t}")

        def ffn(x_sb, x_new, w1_n, w2_n, l, relu_out, with_ln,
                b1_col_base=None, b2_row=None):
            """x_new = [LN](relu(x@w1+b1)@w2 + b2 [+x]); relu_out for the
            subsampling block (no LN, relu on output)."""
            for b in range(BPC):
                xT = transpose_b(x_sb, b, f"xT_ffn_{l}_{b}")
                psy = [psum([P, 1024], "S", f"y2_{l}_{b}_{tp}")
                       for tp in range(NTB // 2)]
                def emit_h1(hc):
                    w1c = pool.tile([P, NF, 512], MMDT, tag="wf", bufs=4,
                                    name=f"w1_{l}_{b}_{hc}")
                    nc.sync.dma_start(
                        w1c[:], wd[w1_n].rearrange("(k p) d -> p k d", p=P)
                        [:, :, hc * 512:(hc + 1) * 512])
                    h1 = pool.tile([P, NF, 512], MMDT, tag="h1", bufs=2,
                                   name=f"h1_{l}_{b}_{hc}")
                    for fp in range(NF // 2):
                        ph = psum([P, 1024], "S", f"h_{l}_{b}_{hc}_{fp}")
                        for j in range(2):
                            f = fp * 2 + j
                            for kt in range(NF):
                                nc.tensor.matmul(
                                    ph[:, j * 512:(j + 1) * 512],
                                    mmv(w1c[:, kt, f * P:(f + 1) * P]),
                                    mmv(xT[:, kt, :]),
                                    start=(kt == 0), stop=(kt == NF - 1))
                        if b1_col_base is not None:
                            for j in range(2):
                                f = fp * 2 + j
                                cb = hc * NF + f
                                nc.scalar.activation(
                                    h1[:, f, :], ph[:, j * 512:(j + 1) * 512],
                                    AF.Relu,
                                    bias=cols_sb[:, b1_col_base + cb:
                                                 b1_col_base + cb + 1])
                        else:
                            nc.vector.tensor_scalar_max(
                                h1[:, fp * 2:fp * 2 + 2, :],
                                ph.rearrange("p (j q) -> p j q", q=512), 0.0)
                    return h1

                def emit_y2(hc, h1):
                    w2c = pool.tile([P, NF, 512], MMDT, tag="wf", bufs=4,
                                    name=f"w2_{l}_{b}_{hc}")
                    nc.sync.dma_start(
                        w2c[:], wd[w2_n].rearrange("(c p) d -> p c d", p=P)
                        [:, hc * NF:(hc + 1) * NF, :])
                    for tp in range(NTB // 2):
                        for j2 in range(2):
                            t = tp * 2 + j2
                            half = psy[tp][:, j2 * 512:(j2 + 1) * 512]
                            for j in range(NF):
                                st = (hc == 0 and j == 0)
                                if st and b2_row is not None:
                                    st = seed_or_start(half, b2_row, 0)
                                nc.tensor.matmul(
                                    half, mmv(h1[:, j, t * P:(t + 1) * P]),
                                    mmv(w2c[:, j, :]),
                                    start=st,
                                    stop=(hc == NHC - 1 and j == NF - 1))

                # software-pipelined: h1(hc+1) issued before y2(hc)
                prev_h1 = None
                for hc in range(NHC):
                    h1c = emit_h1(hc)
                    if prev_h1 is not None:
                        emit_y2(hc - 1, prev_h1)
                    prev_h1 = h1c
                emit_y2(NHC - 1, prev_h1)
                for tp in range(NTB // 2):
                    for j2 in range(2):
                        t = tp * 2 + j2
                        tt = b * NTB + t
                        half = psy[tp][:, j2 * 512:(j2 + 1) * 512]
                        if relu_out:
                            nc.scalar.activation(x_new[:, tt, :], half, AF.Relu)
                        elif with_ln:
                            ln_chain(half, x_sb, x_new, tt, f"ff_{l}",
                                     f"lnf_{l}_{b}_{t}")
                        else:
                            nc.scalar.copy(x_new[:, tt, :], half)

        # ---------------- program ----------------------------------------
        x = pool.tile([P, NT, D], F32, tag="x", bufs=2, name="x_in")
        nc.sync.dma_start(x[:], x0_d.rearrange("(t p) d -> p t d", p=P))

        x1 = pool.tile([P, NT, D], F32, tag="x", bufs=2, name="x_sub")
        ffn(x, x1, "sub_w1", "sub_w2", "sub", relu_out=True, with_ln=False,
            b1_col_base=fl.get("b1_sub"), b2_row=fl.get("b2_sub"))
        x = x1

        for l in range(L):
            xa = pool.tile([P, NT, D], F32, tag="x", bufs=2, name=f"x_sa_{l}")
            attention(x, xa, f"sa_wq_{l}", f"sa_wk_{l}", f"sa_wv_{l}",
                      f"sa_wo_{l}", cross=False, l=l)
            xb = pool.tile([P, NT, D], F32, tag="x", bufs=2, name=f"x_ca_{l}")
            attention(xa, xb, f"ca_wq_{l}", f"ca_wk_{l}", f"ca_wv_{l}",
                      f"ca_wo_{l}", cross=True, l=l)
            xc = pool.tile([P, NT, D], F32, tag="x", bufs=2, name=f"x_ff_{l}")
            ffn(xb, xc, f"ff_w1_{l}", f"ff_w2_{l}", l, relu_out=False,
                with_ln=True, b1_col_base=fl.get(f"b1_ff_{l}"),
                b2_row=fl.get(f"b2_ff_{l}"))
            x = xc

        nc.sync.dma_start(out_d.rearrange("(t p) d -> p t d", p=P), x[:])

        ppool.release(); spool.release(); pool.release(); cpool.release()

    _split_multi_waits(nc)

    names = ["x0", "ident", "ones", "sel8", "encT", "causalT", "keep_dec",
             "keep_enc"] + wname
    return nc, names


# -------------------------------------------------------------------- host
def _host_prep(inputs):
    """Returns (per-core input maps, build flags)."""
    npa = {k: np.asarray(v) for k, v in inputs.items()}
    dec = npa["dec_inputs"]          # [B, S] int
    enc_in = npa["enc_inputs"]       # [B, S] int
    enc_out = np.ascontiguousarray(npa["enc_outputs"], dtype=np.float32)
    pad = int(npa["pad_ids"])
    emb = npa["emb"].astype(np.float32, copy=False)
    pe = npa["pe"].astype(np.float32, copy=False)

    x0 = emb[dec] + pe[None, :S]                       # [B, S, D]
    x0 = np.ascontiguousarray(x0, dtype=np.float32)

    causal = np.where(
        np.arange(S)[None, :] >= np.arange(S)[:, None], 0.0, -1e9
    ).astype(np.float32)                               # [k, q]
    import ml_dtypes
    causalT = np.ascontiguousarray(
        causal.reshape(NTB, P, S).transpose(1, 0, 2)).astype(ml_dtypes.bfloat16)

    keep_dec = (dec != pad).astype(np.float32)         # [B, S]
    keep_enc = (enc_in != pad).astype(np.float32)

    flags = {}
    sel8 = np.zeros((P, 2 * P), dtype=np.float32)
    for j in range(2):
        sel8[64 * j, j * P:j * P + 64] = 1.0
        sel8[64 * j + 32, j * P + 64:j * P + P] = 1.0
    shared = {"causalT": causalT,
              "ident": np.eye(P, dtype=np.float32),
              "ones": np.ones((1, P), dtype=np.float32),
              "sel8": sel8}
    shared["sub_w1"] = np.ascontiguousarray(npa["sub_w1"], dtype=np.float32)
    shared["sub_w2"] = np.ascontiguousarray(npa["sub_w2"], dtype=np.float32)
    for l in range(L):
        shared[f"sa_wq_{l}"] = np.ascontiguousarray(
            npa["sa_wq"][l] / np.sqrt(DK), dtype=np.float32)
        shared[f"ca_wq_{l}"] = np.ascontiguousarray(
            npa["ca_wq"][l] / np.sqrt(DK), dtype=np.float32)
        for pre in ("sa", "ca"):
            for wn in ("wk", "wv", "wo"):
                shared[f"{pre}_{wn}_{l}"] = np.ascontiguousarray(
                    npa[f"{pre}_{wn}"][l], dtype=np.float32)
        shared[f"ff_w1_{l}"] = np.ascontiguousarray(npa["ff_w1"][l],
                                                    dtype=np.float32)
        shared[f"ff_w2_{l}"] = np.ascontiguousarray(npa["ff_w2"][l],
                                                    dtype=np.float32)

    # ---- optional bias / gain handling (all trivial for this model's
    # setup_inputs, so normally nothing extra is emitted) ------------------
    rows, cols = [], []

    def add_row(arr, key):
        if np.any(arr != 0.0):
            flags[key] = len(rows)
            rows.append(np.asarray(arr, dtype=np.float32))

    def add_cols(arr, key):
        if np.any(arr != 0.0):
            flags[key] = len(cols)
            c = np.asarray(arr, dtype=np.float32).reshape(-1, P).T  # [P, n]
            for i in range(c.shape[1]):
                cols.append(c[:, i])

    def add_gain(g_arr, b_arr, step):
        if np.any(g_arr != 1.0):
            flags[f"g_{step}"] = len(rows)
            rows.append(np.asarray(g_arr, dtype=np.float32))
        if np.any(b_arr != 0.0):
            flags[f"beta_{step}"] = len(rows)
            rows.append(np.asarray(b_arr, dtype=np.float32))

    add_cols(npa["sub_b1"], "b1_sub")
    add_row(npa["sub_b2"], "b2_sub")
    for l in range(L):
        add_cols(npa["sa_bq"][l] / np.sqrt(DK), f"bq_self_{l}")
        add_cols(npa["sa_bk"][l], f"bk_self_{l}")
        add_row(npa["sa_bv"][l], f"bv_self_{l}")
        add_row(npa["sa_bo"][l], f"bo_self_{l}")
        add_gain(npa["sa_g"][l], npa["sa_beta"][l], f"self_{l}")
        add_cols(npa["ca_bq"][l] / np.sqrt(DK), f"bq_cross_{l}")
        add_cols(npa["ca_bk"][l], f"bk_cross_{l}")
        add_row(npa["ca_bv"][l], f"bv_cross_{l}")
        add_row(npa["ca_bo"][l], f"bo_cross_{l}")
        add_gain(npa["ca_g"][l], npa["ca_beta"][l], f"cross_{l}")
        add_cols(npa["ff_b1"][l], f"b1_ff_{l}")
        add_row(npa["ff_b2"][l], f"b2_ff_{l}")
        add_gain(npa["ff_g"][l], npa["ff_beta"][l], f"ff_{l}")
    if rows:
        flags["any_rows"] = True
        flags["n_rows"] = len(rows)
        shared["rows"] = np.stack(rows)
    if cols:
        flags["any_cols"] = True
        flags["n_cols"] = len(cols)
        shared["cols"] = np.ascontiguousarray(np.stack(cols, axis=1))

    in_maps = []
    for c in range(NCORES):
        bs = slice(c * BPC, (c + 1) * BPC)
        m = dict(shared)
        m["x0"] = x0[bs].reshape(T, D)
        e = enc_out[bs].reshape(T, D)                      # [T, D]
        m["encT"] = np.ascontiguousarray(
            e.T.reshape(NF, P, T).transpose(1, 0, 2))      # [P, NF, T]
        m["keep_dec"] = np.ascontiguousarray(
            keep_dec[bs].reshape(NT, P).T)                 # [P, NT]
        m["keep_enc"] = np.ascontiguousarray(
            keep_enc[bs].reshape(NT, P).T)
        in_maps.append(m)
    return in_maps, flags


_cache = {}


def run(inputs, mm_dt="f32r", trace=False):
    """Build (cached), run on 8 cores, gather.  Returns (out, results)."""
    _ensure_env()
    from concourse.bass_utils import run_bass_kernel_spmd

    in_maps, flags = _host_prep(inputs)
    key = (mm_dt, tuple(sorted(flags.items())))
    if key not in _cache:
        _cache[key] = _build(mm_dt=mm_dt, flags=flags)
    nc, names = _cache[key]
    res = run_bass_kernel_spmd(nc, in_maps, core_ids=list(range(NCORES)),
                               trace=trace)
    out = np.stack([r["out"] for r in res.results])        # [8, T, D]
    out = out.reshape(B, S, D)
    return out, res


def kernel(**inputs) -> np.ndarray:
    out, _ = run(inputs, mm_dt="f32r", trace=False)
    return out


# revision 30
# speedup vs baseline: 1.0442x; 1.0442x over previous
"""Trainium2 Bass kernel for a 6-layer transformer decoder (nn_Decoder).

Sharding: data-parallel over batch across 8 NeuronCores (2 sequences/core,
weights replicated).  Everything heavy (subsampling FFN, 6 decoder layers:
self-attn, cross-attn, FFN, layernorms, softmaxes) runs on device; the host
only does the embedding-table gather + positional-encoding add, computes the
tiny pad-keep masks / causal mask constants, and folds the 1/sqrt(dk) scale
into the query projection weights.

Device-side layout strategy:
  - activations kept in BOTH token-major [tok, D] (for residual + layernorm)
    and feature-major transposed [D, tok] (as matmul contraction operands).
  - attention computed with TRANSPOSED scores s^T[k, q] so the softmax
    numerator (exp) needs no transposes at all; the softmax denominator is
    produced by an extra ones-column appended to V in the ctx matmul; pad
    masking is applied by zeroing V rows (and the ones column) with a
    per-partition keep-scale during the V copy; causal masking is one
    vector-engine add of a resident causal^T tile.
"""

import sys

import numpy as np

# ---------------------------------------------------------------- constants
B, S, D, H, FF, L, V = 16, 512, 512, 8, 2048, 6, 32000
NCORES = 8
BPC = B // NCORES          # batches per core
T = BPC * S                # tokens per core
P = 128
DK = D // H                # 64
NT = T // P                # 8 token tiles per core
NTB = S // P               # 4 token tiles per batch
NF = D // P                # 4 feature tiles
NHC = 4                    # hidden chunks in FFN (each FF/NHC = 512 wide)
EPS = 1e-5

_REPO = "/opt/trn_rl_repo"


def _ensure_env():
    if _REPO not in sys.path:
        sys.path.insert(0, _REPO)
    import concourse.mybir as mybir
    import concourse.tile as tile
    from bass_rust import ScopedClock

    # This container's walrus build only accepts ONE sync-wait per CTRL
    # instruction; the stock TileContext tail drain aggregates all residual
    # clock waits onto a single Drain.  Split them across multiple drains.
    if not getattr(tile.TileContext, "_ant_drain_patched", False):

        def _drain_and_barrier(self, tick_clock, wait_clock):
            nc = self.nc
            probe = nc.sync.drain()
            wait_clock.add_sem_waits(
                probe.ins, ScopedClock({None: tick_clock.global_clock})
            )
            si = probe.ins.sync_info
            waits = list(si.on_wait) if si is not None else []
            if len(waits) > 1:
                probe.ins.sync_info = mybir.SyncInfo(
                    on_wait=[waits[0]], on_update=list(si.on_update)
                )
                for w in waits[1:]:
                    extra = nc.sync.drain()
                    extra.ins.sync_info = mybir.SyncInfo(on_wait=[w], on_update=[])
            nc.all_engine_barrier()
            popped = nc._tile_sem_poison_stack.pop()
            assert popped is self._sem_poison
            nc.clear_and_free_semaphores(list(self.sems.allocated().values()))
            nc.all_engine_barrier()

        tile.TileContext._drain_and_barrier = _drain_and_barrier
        tile.TileContext._ant_drain_patched = True


def _split_multi_waits(nc):
    """This container's walrus build accepts only ONE sync-wait per
    instruction.  Hoist extra waits onto single-wait NoOps inserted just
    before the instruction on the same engine (per-engine program order is
    what the sequencers execute, so the gating semantics are identical)."""
    import concourse.mybir as mybir

    n = 0
    for f in nc.m.functions:
        for b in f.blocks:
            insts = list(b.instructions)
            out = []
            changed = False
            for inst in insts:
                si = inst.sync_info
                if si is not None and len(si.on_wait) > 1:
                    waits = list(si.on_wait)
                    for k, w in enumerate(waits[:-1]):
                        nop = mybir.InstNoOp(name=f"{inst.name}_sw{k}",
                                             engine=inst.engine, ins=[], outs=[])
                        nop.sync_info = mybir.SyncInfo(on_wait=[w], on_update=[])
                        out.append(nop)
                        n += 1
                    inst.sync_info = mybir.SyncInfo(
                        on_wait=[waits[-1]], on_update=list(si.on_update))
                    changed = True
                out.append(inst)
            if changed:
                b.instructions = out
    return n


# ------------------------------------------------------------------ builder
def _build(mm_dt="f32r", flags=None):
    """Emit the Bass program.  Returns (nc, input_names)."""
    _ensure_env()
    import concourse.bass as bass
    import concourse.mybir as mybir
    import concourse.tile as tile
    fl = flags or {}
    F32 = mybir.dt.float32
    # float32r = single-pass fp32 matmul mode (4x faster than fp32 on the PE
    # at N>=512, TF32-like rounding of the multiplicands, fp32 accumulate).
    # The BIR verifier requires every producer of an fp32r matmul operand to
    # declare fp32r output, so matmul-feeding tensors are typed MMDT
    # end-to-end.
    MMDT = mybir.dt.float32r if mm_dt == "f32r" else F32

    def mmv(ap):
        return ap

    nc = bass.Bass()

    # ---------------- dram tensors
    x0_d = nc.dram_tensor("x0", [T, D], F32, kind="ExternalInput")
    ident_d = nc.dram_tensor("ident", [P, P], F32, kind="ExternalInput")
    ones_d = nc.dram_tensor("ones", [1, P], MMDT, kind="ExternalInput")
    sel8_d = nc.dram_tensor("sel8", [P, 2 * P], MMDT, kind="ExternalInput")
    encT_d = nc.dram_tensor("encT", [P, NF, T], MMDT, kind="ExternalInput")
    causal_d = nc.dram_tensor("causalT", [P, NTB, S], mybir.dt.bfloat16,
                              kind="ExternalInput")
    keep_dec_d = nc.dram_tensor("keep_dec", [P, NT], F32, kind="ExternalInput")
    keep_enc_d = nc.dram_tensor("keep_enc", [P, NT], F32, kind="ExternalInput")
    out_d = nc.dram_tensor("out", [T, D], F32, kind="ExternalOutput")

    wname = []
    wd = {}

    def dram_w(name, shape):
        wd[name] = nc.dram_tensor(name, shape, MMDT, kind="ExternalInput")
        wname.append(name)

    dram_w("sub_w1", [D, FF]); dram_w("sub_w2", [FF, D])
    for l in range(L):
        for pre in ("sa", "ca"):
            for wn in ("wq", "wk", "wv", "wo"):
                dram_w(f"{pre}_{wn}_{l}", [D, D])
        dram_w(f"ff_w1_{l}", [D, FF]); dram_w(f"ff_w2_{l}", [FF, D])
    if fl.get("any_rows"):
        dram_w("rows", [fl["n_rows"], D])
    if fl.get("any_cols"):
        wd["cols"] = nc.dram_tensor("cols", [P, fl["n_cols"]], F32, kind="ExternalInput")
        wname.append("cols")

    AF = mybir.ActivationFunctionType
    OP = mybir.AluOpType

    with tile.TileContext(nc) as tc:
        cpool = tc.alloc_tile_pool(name="consts", bufs=1)
        pool = tc.alloc_tile_pool(name="work", bufs=2)
        spool = tc.alloc_tile_pool(name="stats", bufs=8)
        ppool = tc.alloc_tile_pool(name="ps", bufs=2, space="PSUM")

        ident = cpool.tile([P, P], F32, name="ident_sb")
        nc.sync.dma_start(ident[:], ident_d[:])
        causalT = cpool.tile([P, NTB, S], mybir.dt.bfloat16, name="causalT")
        nc.sync.dma_start(causalT[:], causal_d[:])
        keep_dec = cpool.tile([P, NT], F32, name="keep_dec_sb")
        nc.sync.dma_start(keep_dec[:], keep_dec_d[:])
        keep_enc = cpool.tile([P, NT], F32, name="keep_enc_sb")
        nc.sync.dma_start(keep_enc[:], keep_enc_d[:])
        ones_row = cpool.tile([1, P], MMDT, name="ones_row")
        nc.sync.dma_start(ones_row[:], ones_d[:])
        sel8 = cpool.tile([P, 2 * P], MMDT, name="sel8_sb")
        nc.sync.dma_start(sel8[:], sel8_d[:])
        eps_col = cpool.tile([P, 1], F32, name="eps_col")
        nc.vector.memset(eps_col[:], EPS)
        rows_sb = cols_sb = None
        if fl.get("any_rows"):
            rows_sb = cpool.tile([fl["n_rows"], D], MMDT, name="rows_sb")
            nc.sync.dma_start(rows_sb[:], wd["rows"][:])
        if fl.get("any_cols"):
            cols_sb = cpool.tile([P, fl["n_cols"]], F32, name="cols_sb")
            nc.sync.dma_start(cols_sb[:], wd["cols"][:])

        def load_w_proj(name):
            w = pool.tile([P, NF, D], MMDT, tag="w", bufs=3, name="w_" + name)
            nc.sync.dma_start(w[:], wd[name].rearrange("(k p) d -> p k d", p=P))
            return w

        def psum(shape, tag, name):
            return ppool.tile(shape, F32, tag=tag,
                              bufs=(3 if tag == "S" else 2), name=name)

        def seed_or_start(ps, row_idx, kt):
            """Start flag for the kt-th accumulation matmul; optionally seed
            the psum with a broadcast bias row first (K=1 matmul)."""
            if row_idx is not None and kt == 0:
                nc.tensor.matmul(
                    ps, mmv(ones_row[0:1, 0:ps.shape[0]]),
                    mmv(rows_sb[row_idx:row_idx + 1, :ps.shape[-1]]),
                    start=True, stop=False)
                return False
            return kt == 0

        # ---------------- building blocks --------------------------------
        def transpose_b(x_sb, b, name):
            """token-major x tile [P, NT, D] (batch b) -> feature-major
            xT_b [P, NF, S]."""
            xT = pool.tile([P, NF, S], MMDT, tag="xT", bufs=2, name=name)
            for t in range(NTB):
                ps = psum([P, 512], "C", f"{name}_ps{t}")
                for f in range(NF):
                    nc.tensor.transpose(
                        ps[:, f * P:(f + 1) * P],
                        x_sb[:, b * NTB + t, f * P:(f + 1) * P],
                        ident[:],
                    )
                nc.vector.tensor_copy(
                    out=xT[:, :, t * P:(t + 1) * P],
                    in_=ps.rearrange("p (f j) -> p f j", j=P),
                )
            return xT

        def proj_featmajor(srcT, w, tag, name, col_base=None):
            """y^T = (src @ w)^T: [P, NF, S] feature-major output.  Two
            output feature tiles share one 2-bank psum for deeper PE
            pipelining and halved copy-out count."""
            yT = pool.tile([P, NF, S], MMDT, tag=tag, bufs=1, name=name)
            for fp in range(NF // 2):
                ps = psum([P, 1024], "S", f"{name}_ps{fp}")
                for j in range(2):
                    f = fp * 2 + j
                    for kt in range(NF):
                        nc.tensor.matmul(
                            ps[:, j * 512:(j + 1) * 512],
                            mmv(w[:, kt, f * P:(f + 1) * P]), mmv(srcT[:, kt, :]),
                            start=(kt == 0), stop=(kt == NF - 1))
                if col_base is not None:
                    for j in range(2):
                        f = fp * 2 + j
                        nc.scalar.activation(
                            yT[:, f, :], ps[:, j * 512:(j + 1) * 512], AF.Identity,
                            bias=cols_sb[:, col_base + f:col_base + f + 1])
                else:
                    nc.scalar.copy(yT[:, fp * 2:fp * 2 + 2, :],
                                   ps.rearrange("p (j q) -> p j q", q=512))
            return yT

        def proj_v(srcT, w, keep, b, name, row_idx=None):
            """token-major V with interleaved ones-columns, rows scaled by
            keep (pad masking).  [P, NTB, H*65] for batch b."""
            v = pool.tile([P, NTB, H * 65], MMDT, tag="v", bufs=1, name=name)
            for t in range(NTB):
                ps = psum([P, 512], "C", f"{name}_ps{t}")
                for kt in range(NF):
                    st = seed_or_start(ps, row_idx, kt)
                    nc.tensor.matmul(
                        ps, mmv(srcT[:, kt, t * P:(t + 1) * P]), mmv(w[:, kt, :]),
                        start=st, stop=(kt == NF - 1))
                kcol = keep[:, b * NTB + t:b * NTB + t + 1]
                vv = v[:, t, :].rearrange("p (h c) -> p h c", c=65)
                nc.scalar.activation(vv[:, :, 0:64],
                                     ps.rearrange("p (h c) -> p h c", c=64),
                                     AF.Copy, scale=kcol)
                nc.vector.tensor_copy(out=vv[:, :, 64:65],
                                      in_=kcol.to_broadcast((P, H, 1)))
            return v

        def ln_chain(ps, x_old, x_new, tt, step, name):
            """x_new[tt] = LN(ps + x_old[tt]) with fused reductions."""
            ssum = spool.tile([P, 1], F32, tag="st", bufs=8, name=name + "_s")
            nc.vector.tensor_add(out=x_new[:, tt, :], in0=ps,
                                 in1=x_old[:, tt, :])
            # row-sum for the mean, riding along an ACT copy into the (dead)
            # psum tile
            nc.scalar.activation(ps, x_new[:, tt, :], AF.Copy,
                                 accum_out=ssum[:])
            negm = spool.tile([P, 1], F32, tag="st", bufs=8, name=name + "_m")
            nc.scalar.mul(negm[:], ssum[:], -1.0 / D)
            var = spool.tile([P, 1], F32, tag="st", bufs=8, name=name + "_v")
            nc.scalar.activation(ps, x_new[:, tt, :], AF.Square,
                                 bias=negm[:], accum_out=var[:])
            std = spool.tile([P, 1], F32, tag="st", bufs=8, name=name + "_d")
            nc.scalar.activation(std[:], var[:], AF.Sqrt,
                                 bias=eps_col[:], scale=1.0 / D)
            rstd = spool.tile([P, 1], F32, tag="st", bufs=8, name=name + "_r")
            nc.vector.reciprocal(rstd[:], std[:])
            nc.vector.tensor_scalar(
                out=x_new[:, tt, :], in0=x_new[:, tt, :],
                scalar1=negm[:], scalar2=rstd[:], op0=OP.add, op1=OP.mult)
            gi = fl.get(f"g_{step}")
            if gi is not None:
                gb = _row_bcast(gi, name + "_g")
                nc.vector.tensor_mul(out=x_new[:, tt, :],
                                     in0=x_new[:, tt, :], in1=gb[0:P, :])
            bi = fl.get(f"beta_{step}")
            if bi is not None:
                bb = _row_bcast(bi, name + "_b")
                nc.vector.tensor_add(out=x_new[:, tt, :],
                                     in0=x_new[:, tt, :], in1=bb[0:P, :])

        def _row_bcast(row_idx, name):
            ps = psum([P, 512], "C", name + "_ps")
            nc.tensor.matmul(ps, mmv(ones_row[0:1, 0:P]),
                             mmv(rows_sb[row_idx:row_idx + 1, :]),
                             start=True, stop=True)
            sb = pool.tile([P, 512], F32, tag="gb", bufs=2, name=name + "_sb")
            nc.scalar.copy(sb[:], ps)
            return sb

        def attention(x_sb, x_new, wq_n, wk_n, wv_n, wo_n, cross, l):
            """One attention sub-block (self or cross) + residual + LN."""
            tagp = "cross" if cross else "self"
            wq = load_w_proj(wq_n); wk = load_w_proj(wk_n)
            wv = load_w_proj(wv_n)
            keep = keep_enc if cross else keep_dec
            D_qT, D_kT, D_vv, D_ctxT, D_dns = {}, {}, {}, {}, {}

            def emit_prep(b):
                xT = transpose_b(x_sb, b, f"xT_{tagp}_{l}_{b}")
                if cross:
                    srcT = pool.tile([P, NF, S], MMDT, tag="encT", bufs=1,
                                     name=f"encTs_{l}_{b}")
                    nc.sync.dma_start(srcT[:], encT_d[:, :, b * S:(b + 1) * S])
                else:
                    srcT = xT
                D_qT[b] = proj_featmajor(xT, wq, "qT", f"qT_{tagp}_{l}_{b}",
                                         col_base=fl.get(f"bq_{tagp}_{l}"))
                D_kT[b] = proj_featmajor(srcT, wk, "kT", f"kT_{tagp}_{l}_{b}",
                                         col_base=fl.get(f"bk_{tagp}_{l}"))
                D_vv[b] = proj_v(srcT, wv, keep, b, f"v_{tagp}_{l}_{b}",
                                 row_idx=fl.get(f"bv_{tagp}_{l}"))
                D_ctxT[b] = pool.tile([P, NF, S], MMDT, tag="ctxT", bufs=2,
                                      name=f"ctxT_{tagp}_{l}_{b}")
                dns = []
                for g in range(2):
                    dng = pool.tile([P, 512], F32, tag="dncol", bufs=4,
                                    name=f"dncol_{tagp}_{l}_{b}_{g}")
                    nc.vector.memset(dng[:], 1.0)
                    dns.append(dng)
                D_dns[b] = dns
            def emit_scores(b, h):
                qT, kT = D_qT[b], D_kT[b]
                po = (h % 2) * 64
                ft = h // 2
                expT = pool.tile([P, NTB, S], MMDT, tag="expT", bufs=2,
                                 name=f"expT_{tagp}_{l}_{b}_{h}")
                for half in range(2):
                    ps = psum([P, 1024], "S", f"s_{tagp}_{l}_{b}_{h}_{half}")
                    for j in range(2):
                        kt = half * 2 + j
                        nc.tensor.matmul(
                            ps[:, j * 512:(j + 1) * 512],
                            mmv(kT[po:po + 64, ft, kt * P:(kt + 1) * P]),
                            mmv(qT[po:po + 64, ft, :]),
                            start=True, stop=True)
                    psv = ps.rearrange("p (j q) -> p j q", q=512)
                    if not cross:
                        nc.vector.tensor_tensor(
                            out=psv, in0=psv,
                            in1=causalT[:, half * 2:half * 2 + 2, :],
                            op=OP.add)
                    nc.scalar.activation(
                        expT[:, half * 2:half * 2 + 2, :], psv, AF.Exp)
                return expT

            ctx_ps = {}

            def emit_ctx(b, h, expT):
                vv, ctxT, dns = D_vv[b], D_ctxT[b], D_dns[b]
                po = (h % 2) * 64
                ft = h // 2
                if h % 2 == 0:
                    ctx_ps[b] = psum([P, 1024], "S", f"c2_{tagp}_{l}_{b}_{h}")
                psc = ctx_ps[b][:, (h % 2) * 512:(h % 2 + 1) * 512]
                for kt in range(NTB):
                    nc.tensor.matmul(
                        psc[0:65, :],
                        mmv(vv[:, kt, h * 65:h * 65 + 65]),
                        mmv(expT[:, kt, :]),
                        start=(kt == 0), stop=(kt == NTB - 1))
                ro = 32 * (h % 4)
                nc.scalar.copy(dns[h // 4][ro:ro + 1, :], psc[64:65, :])
                nc.vector.tensor_copy(out=ctxT[po:po + 64, ft, :],
                                      in_=psc[0:64, :])

            # software-pipelined over (h, b): the in-order PE always has
            # independent scores work while exp/copies of the previous
            # (h, b) pair complete on the scalar/vector engines
            for b in range(BPC):
                emit_prep(b)
                prev = None
                for h in range(H):
                    e = emit_scores(b, h)
                    if prev is not None:
                        emit_ctx(*prev)
                    prev = (b, h, e)
                emit_ctx(*prev)
            # batched reciprocals, head-pair broadcast + in-place normalize
            D_rcs = {}
            for b in range(BPC):
                rcs = []
                for g in range(2):
                    rcg = pool.tile([P, 512], MMDT, tag="rc", bufs=4,
                                    name=f"rc_{tagp}_{l}_{b}_{g}")
                    with nc.allow_low_precision(reason="fp32r matmul feed"):
                        nc.vector.reciprocal(rcg[:], D_dns[b][g][:])
                    rcs.append(rcg)
                D_rcs[b] = rcs
            for b in range(BPC):
                for hp in range(H // 2):
                    psr = psum([P, 512], "C", f"rb_{tagp}_{l}_{b}_{hp}")
                    j = hp % 2
                    nc.tensor.matmul(psr, mmv(sel8[:, j * P:(j + 1) * P]),
                                     mmv(D_rcs[b][hp // 2][:]),
                                     start=True, stop=True)
                    nc.vector.tensor_mul(out=D_ctxT[b][:, hp, :],
                                         in0=D_ctxT[b][:, hp, :],
                                         in1=psr[:])
            # output projection + residual + LN, token-tile pairs
            wo = load_w_proj(wo_n)
            row_idx = fl.get(f"bo_{tagp}_{l}")
            for b in range(BPC):
                ctxT = D_ctxT[b]
                for tp in range(NTB // 2):
                    ps = psum([P, 1024], "S", f"o_{tagp}_{l}_{b}_{tp}")
                    for j in range(2):
                        t = tp * 2 + j
                        half = ps[:, j * 512:(j + 1) * 512]
                        for ft2 in range(NF):
                            st = seed_or_start(half, row_idx, ft2)
                            nc.tensor.matmul(
                                half, mmv(ctxT[:, ft2, t * P:(t + 1) * P]),
                                mmv(wo[:, ft2, :]),
                                start=st, stop=(ft2 == NF - 1))
                    for j in range(2):
                        t = tp * 2 + j
                        ln_chain(ps[:, j * 512:(j + 1) * 512], x_sb, x_new,
                                 b * NTB + t, f"{tagp}_{l}",
                                 f"ln_{tagp}_{l}_{b}_{t}")

        def ffn(x_sb, x_new, w1_n, w2_n, l, relu_out, with_ln,
                b1_col_base=None, b2_row=None):
            """x_new = [LN](relu(x@w1+b1)@w2 + b2 [+x]); relu_out for the
            subsampling block (no LN, relu on output)."""
            for b in range(BPC):
                xT = transpose_b(x_sb, b, f"xT_ffn_{l}_{b}")
                psy = [psum([P, 1024], "S", f"y2_{l}_{b}_{tp}")
                       for tp in range(NTB // 2)]
                def emit_h1(hc):
                    w1c = pool.tile([P, NF, 512], MMDT, tag="wf", bufs=4,
                                    name=f"w1_{l}_{b}_{hc}")
                    nc.sync.dma_start(
                        w1c[:], wd[w1_n].rearrange("(k p) d -> p k d", p=P)
                        [:, :, hc * 512:(hc + 1) * 512])
                    h1 = pool.tile([P, NF, 512], MMDT, tag="h1", bufs=2,
                                   name=f"h1_{l}_{b}_{hc}")
                    for fp in range(NF // 2):
                        ph = psum([P, 1024], "S", f"h_{l}_{b}_{hc}_{fp}")
                        for j in range(2):
                            f = fp * 2 + j
                            for kt in range(NF):
                                nc.tensor.matmul(
                                    ph[:, j * 512:(j + 1) * 512],
                                    mmv(w1c[:, kt, f * P:(f + 1) * P]),
                                    mmv(xT[:, kt, :]),
                                    start=(kt == 0), stop=(kt == NF - 1))
                        if b1_col_base is not None:
                            for j in range(2):
                                f = fp * 2 + j
                                cb = hc * NF + f
                                nc.scalar.activation(
                                    h1[:, f, :], ph[:, j * 512:(j + 1) * 512],
                                    AF.Relu,
                                    bias=cols_sb[:, b1_col_base + cb:
                                                 b1_col_base + cb + 1])
                        else:
                            nc.vector.tensor_scalar_max(
                                h1[:, fp * 2:fp * 2 + 2, :],
                                ph.rearrange("p (j q) -> p j q", q=512), 0.0)
                    return h1

                def emit_y2(hc, h1):
                    w2c = pool.tile([P, NF, 512], MMDT, tag="wf", bufs=4,
                                    name=f"w2_{l}_{b}_{hc}")
                    nc.sync.dma_start(
                        w2c[:], wd[w2_n].rearrange("(c p) d -> p c d", p=P)
                        [:, hc * NF:(hc + 1) * NF, :])
                    for tp in range(NTB // 2):
                        for j2 in range(2):
                            t = tp * 2 + j2
                            half = psy[tp][:, j2 * 512:(j2 + 1) * 512]
                            for j in range(NF):
                                st = (hc == 0 and j == 0)
                                if st and b2_row is not None:
                                    st = seed_or_start(half, b2_row, 0)
                                nc.tensor.matmul(
                                    half, mmv(h1[:, j, t * P:(t + 1) * P]),
                                    mmv(w2c[:, j, :]),
                                    start=st,
                                    stop=(hc == NHC - 1 and j == NF - 1))

                # software-pipelined: h1(hc+1) issued before y2(hc)
                prev_h1 = None
                for hc in range(NHC):
                    h1c = emit_h1(hc)
                    if prev_h1 is not None:
                        emit_y2(hc - 1, prev_h1)
                    prev_h1 = h1c
                emit_y2(NHC - 1, prev_h1)
                for tp in range(NTB // 2):
                    for j2 in range(2):
                        t = tp * 2 + j2
                        tt = b * NTB + t
                        half = psy[tp][:, j2 * 512:(j2 + 1) * 512]
                        if relu_out:
                            nc.scalar.activation(x_new[:, tt, :], half, AF.Relu)
                        elif with_ln:
                            ln_chain(half, x_sb, x_new, tt, f"ff_{l}",
                                     f"lnf_{l}_{b}_{t}")
                        else:
                            nc.scalar.copy(x_new[:, tt, :], half)

        # ---------------- program ----------------------------------------
        x = pool.tile([P, NT, D], F32, tag="x", bufs=2, name="x_in")
        nc.sync.dma_start(x[:], x0_d.rearrange("(t p) d -> p t d", p=P))

        x1 = pool.tile([P, NT, D], F32, tag="x", bufs=2, name="x_sub")
        ffn(x, x1, "sub_w1", "sub_w2", "sub", relu_out=True, with_ln=False,
            b1_col_base=fl.get("b1_sub"), b2_row=fl.get("b2_sub"))
        x = x1

        for l in range(L):
            xa = pool.tile([P, NT, D], F32, tag="x", bufs=2, name=f"x_sa_{l}")
            attention(x, xa, f"sa_wq_{l}", f"sa_wk_{l}", f"sa_wv_{l}",
                      f"sa_wo_{l}", cross=False, l=l)
            xb = pool.tile([P, NT, D], F32, tag="x", bufs=2, name=f"x_ca_{l}")
            attention(xa, xb, f"ca_wq_{l}", f"ca_wk_{l}", f"ca_wv_{l}",
                      f"ca_wo_{l}", cross=True, l=l)
            xc = pool.tile([P, NT, D], F32, tag="x", bufs=2, name=f"x_ff_{l}")
            ffn(xb, xc, f"ff_w1_{l}", f"ff_w2_{l}", l, relu_out=False,
                with_ln=True, b1_col_base=fl.get(f"b1_ff_{l}"),
                b2_row=fl.get(f"b2_ff_{l}"))
            x = xc

        nc.sync.dma_start(out_d.rearrange("(t p) d -> p t d", p=P), x[:])

        ppool.release(); spool.release(); pool.release(); cpool.release()

    _split_multi_waits(nc)

    names = ["x0", "ident", "ones", "sel8", "encT", "causalT", "keep_dec",
             "keep_enc"] + wname
    return nc, names


# -------------------------------------------------------------------- host
def _host_prep(inputs):
    """Returns (per-core input maps, build flags)."""
    npa = {k: np.asarray(v) for k, v in inputs.items()}
    dec = npa["dec_inputs"]          # [B, S] int
    enc_in = npa["enc_inputs"]       # [B, S] int
    enc_out = np.ascontiguousarray(npa["enc_outputs"], dtype=np.float32)
    pad = int(npa["pad_ids"])
    emb = npa["emb"].astype(np.float32, copy=False)
    pe = npa["pe"].astype(np.float32, copy=False)

    x0 = emb[dec] + pe[None, :S]                       # [B, S, D]
    x0 = np.ascontiguousarray(x0, dtype=np.float32)

    causal = np.where(
        np.arange(S)[None, :] >= np.arange(S)[:, None], 0.0, -1e9
    ).astype(np.float32)                               # [k, q]
    import ml_dtypes
    causalT = np.ascontiguousarray(
        causal.reshape(NTB, P, S).transpose(1, 0, 2)).astype(ml_dtypes.bfloat16)

    keep_dec = (dec != pad).astype(np.float32)         # [B, S]
    keep_enc = (enc_in != pad).astype(np.float32)

    flags = {}
    sel8 = np.zeros((P, 2 * P), dtype=np.float32)
    for j in range(2):
        sel8[64 * j, j * P:j * P + 64] = 1.0
        sel8[64 * j + 32, j * P + 64:j * P + P] = 1.0
    shared = {"causalT": causalT,
              "ident": np.eye(P, dtype=np.float32),
              "ones": np.ones((1, P), dtype=np.float32),
              "sel8": sel8}
    shared["sub_w1"] = np.ascontiguousarray(npa["sub_w1"], dtype=np.float32)
    shared["sub_w2"] = np.ascontiguousarray(npa["sub_w2"], dtype=np.float32)
    for l in range(L):
        shared[f"sa_wq_{l}"] = np.ascontiguousarray(
            npa["sa_wq"][l] / np.sqrt(DK), dtype=np.float32)
        shared[f"ca_wq_{l}"] = np.ascontiguousarray(
            npa["ca_wq"][l] / np.sqrt(DK), dtype=np.float32)
        for pre in ("sa", "ca"):
            for wn in ("wk", "wv", "wo"):
                shared[f"{pre}_{wn}_{l}"] = np.ascontiguousarray(
                    npa[f"{pre}_{wn}"][l], dtype=np.float32)
        shared[f"ff_w1_{l}"] = np.ascontiguousarray(npa["ff_w1"][l],
                                                    dtype=np.float32)
        shared[f"ff_w2_{l}"] = np.ascontiguousarray(npa["ff_w2"][l],
                                                    dtype=np.float32)

    # ---- optional bias / gain handling (all trivial for this model's
    # setup_inputs, so normally nothing extra is emitted) ------------------
    rows, cols = [], []

    def add_row(arr, key):
        if np.any(arr != 0.0):
            flags[key] = len(rows)
            rows.append(np.asarray(arr, dtype=np.float32))

    def add_cols(arr, key):
        if np.any(arr != 0.0):
            flags[key] = len(cols)
            c = np.asarray(arr, dtype=np.float32).reshape(-1, P).T  # [P, n]
            for i in range(c.shape[1]):
                cols.append(c[:, i])

    def add_gain(g_arr, b_arr, step):
        if np.any(g_arr != 1.0):
            flags[f"g_{step}"] = len(rows)
            rows.append(np.asarray(g_arr, dtype=np.float32))
        if np.any(b_arr != 0.0):
            flags[f"beta_{step}"] = len(rows)
            rows.append(np.asarray(b_arr, dtype=np.float32))

    add_cols(npa["sub_b1"], "b1_sub")
    add_row(npa["sub_b2"], "b2_sub")
    for l in range(L):
        add_cols(npa["sa_bq"][l] / np.sqrt(DK), f"bq_self_{l}")
        add_cols(npa["sa_bk"][l], f"bk_self_{l}")
        add_row(npa["sa_bv"][l], f"bv_self_{l}")
        add_row(npa["sa_bo"][l], f"bo_self_{l}")
        add_gain(npa["sa_g"][l], npa["sa_beta"][l], f"self_{l}")
        add_cols(npa["ca_bq"][l] / np.sqrt(DK), f"bq_cross_{l}")
        add_cols(npa["ca_bk"][l], f"bk_cross_{l}")
        add_row(npa["ca_bv"][l], f"bv_cross_{l}")
        add_row(npa["ca_bo"][l], f"bo_cross_{l}")
        add_gain(npa["ca_g"][l], npa["ca_beta"][l], f"cross_{l}")
        add_cols(npa["ff_b1"][l], f"b1_ff_{l}")
        add_row(npa["ff_b2"][l], f"b2_ff_{l}")
        add_gain(npa["ff_g"][l], npa["ff_beta"][l], f"ff_{l}")
    if rows:
        flags["any_rows"] = True
        flags["n_rows"] = len(rows)
        shared["rows"] = np.stack(rows)
    if cols:
        flags["any_cols"] = True
        flags["n_cols"] = len(cols)
        shared["cols"] = np.ascontiguousarray(np.stack(cols, axis=1))

    in_maps = []
    for c in range(NCORES):
        bs = slice(c * BPC, (c + 1) * BPC)
        m = dict(shared)
        m["x0"] = x0[bs].reshape(T, D)
        e = enc_out[bs].reshape(T, D)                      # [T, D]
        m["encT"] = np.ascontiguousarray(
            e.T.reshape(NF, P, T).transpose(1, 0, 2))      # [P, NF, T]
        m["keep_dec"] = np.ascontiguousarray(
            keep_dec[bs].reshape(NT, P).T)                 # [P, NT]
        m["keep_enc"] = np.ascontiguousarray(
            keep_enc[bs].reshape(NT, P).T)
        in_maps.append(m)
    return in_maps, flags


_cache = {}


def run(inputs, mm_dt="f32r", trace=False):
    """Build (cached), run on 8 cores, gather.  Returns (out, results)."""
    _ensure_env()
    from concourse.bass_utils import run_bass_kernel_spmd

    in_maps, flags = _host_prep(inputs)
    key = (mm_dt, tuple(sorted(flags.items())))
    if key not in _cache:
        _cache[key] = _build(mm_dt=mm_dt, flags=flags)
    nc, names = _cache[key]
    res = run_bass_kernel_spmd(nc, in_maps, core_ids=list(range(NCORES)),
                               trace=trace)
    out = np.stack([r["out"] for r in res.results])        # [8, T, D]
    out = out.reshape(B, S, D)
    return out, res


def kernel(**inputs) -> np.ndarray:
    out, _ = run(inputs, mm_dt="f32r", trace=False)
    return out
